# revision 10
# baseline (speedup 1.0000x reference)
"""FAVOR causal self-attention (Performer) Trainium2 kernel.

Sharding: 8 cores = 2 (batch) x 4 (head groups of 4 heads). Each core
computes qkv for its heads, runs chunked linear attention (L=128), applies
its slice of the output projection, and returns a partial (T, C) output;
partials are summed on the host (+ b_proj broadcast).

Math (validated vs the jax reference in numpy):
  per head, Eq = exp(omega.T@qT) (x1/16), EkT = exp(omega.T@kT) (x1/16),
  fk = exp(-||k||^2/2)/sqrt(m):
    A_T[tj,ti] = sum_mm EkT[mm,tj] Eq[mm,ti], masked tj<=ti, row-scaled by fk
    [num|den](ti,:) += EqT_chunk.T @ [S|Z]  +  A_T_m.T @ [V|1]
    [S|Z] += Ek_hat.T @ [V|1] accumulated in PSUM, Ek_hat = exp(projk)*fk/16
  y = num/den; the 1/16 scales cancel in the division (kept for fp16 range).

Layout tricks:
  - k stored per head as ktsq_h (128,T): rows 0:64 = kT, 64:128 = kT^2, so one
    matmul against the host const [omega|0 ; 0|-0.5] yields [projk | -nsq].
  - v stored as (128, 4*65) with a ones column after each head's 64, so the
    intra and state matmuls take a single (tj,65) moving operand.
"""
import math
import sys

sys.path.insert(0, "/opt/trn_rl_repo")

import numpy as np

import concourse.bass as bass
import concourse.mybir as mybir
from concourse.tile import TileContext

T, C = 1024, 1024
NH, D, M = 16, 64, 128
L = 128           # chunk length
HPC = 4           # heads per core
NT = T // 128     # 8 token tiles
NK = C // 128     # 8 contraction tiles
F32, F16 = mybir.dt.float32, mybir.dt.float16
LN_SCALE = math.log(1.0 / 16.0)       # folded into Eq and Ek exps
NEG_HALF_LN_M = -0.5 * math.log(M)


def _split_waits(nc):
    """Walrus codegen accepts 1 sync wait per instruction (2 on
    EventSemaphore). Tile can emit more; hoist the excess onto
    EventSemaphore instructions inserted immediately before, same engine."""
    for fn in nc.m.functions:
        for bb in fn.blocks:
            insts = bb.instructions
            i = 0
            while i < len(insts):
                inst = insts[i]
                si = inst.sync_info
                if si is None:
                    i += 1
                    continue
                waits = list(si.on_wait or [])
                cap = 2 if isinstance(inst, mybir.InstEventSemaphore) else 1
                if len(waits) <= cap:
                    i += 1
                    continue
                keep, excess = waits[:cap], waits[cap:]
                new_insts = []
                for j in range(0, len(excess), 2):
                    ev = mybir.InstEventSemaphore(
                        name=nc.get_next_instruction_name(),
                        engine=inst.engine,
                        ins=[],
                        outs=[],
                        sync_info=mybir.SyncInfo(
                            on_wait=excess[j:j + 2], on_update=[]),
                    )
                    nc.register_instruction(ev)
                    new_insts.append(ev)
                inst.sync_info = mybir.SyncInfo(
                    on_wait=keep, on_update=list(si.on_update or []))
                for k, ev in enumerate(new_insts):
                    insts.insert(i + k, ev)
                i += len(new_insts) + 1


def build_bass():
    nc = bass.Bass()

    xT = nc.dram_tensor("xT", [C, T], F16, kind="ExternalInput")
    wqk = nc.dram_tensor("wqk", [C, 4 * 128], F16, kind="ExternalInput")
    wv = nc.dram_tensor("wv", [C, HPC * D], F16, kind="ExternalInput")
    wp = nc.dram_tensor("wp", [HPC * D, C], F16, kind="ExternalInput")
    omega2 = nc.dram_tensor("omega2", [128, 128], F16, kind="ExternalInput")
    omnsq = nc.dram_tensor("omnsq", [128, 129], F16, kind="ExternalInput")
    maskT = nc.dram_tensor("maskT", [128, 128], F16, kind="ExternalInput")
    ident = nc.dram_tensor("ident", [128, 128], F16, kind="ExternalInput")
    bqk = nc.dram_tensor("bqk", [4 * 128, 1], F32, kind="ExternalInput")
    bv = nc.dram_tensor("bv", [1, HPC * D], F16, kind="ExternalInput")
    outp = nc.dram_tensor("outp", [T, C], F32, kind="ExternalOutput")

    Exp = mybir.ActivationFunctionType.Exp
    Ident = mybir.ActivationFunctionType.Identity

    with TileContext(nc) as tc:
        with (
            tc.tile_pool(name="big", bufs=1) as big,          # resident data
            tc.tile_pool(name="cpy", bufs=3) as cpy,          # staging tiles
            tc.tile_pool(name="chk", bufs=2) as chk,          # chunk tiles
            tc.tile_pool(name="col", bufs=4) as col,          # small columns
            tc.tile_pool(name="ps", bufs=1, space="PSUM") as ps,
        ):
            # PSUM budget (8 banks): bankA x2, pk x1, pA x1, pYt x2, psS x2.
            def bankA():
                return ps.tile([128, 512], F32, name="bankA", bufs=2)

            # ---- load resident inputs (small consts first) ----
            om_sb = big.tile([128, 128], F16, name="om")
            nc.sync.dma_start(out=om_sb, in_=omega2[:, :])
            on_sb = big.tile([128, 129], F16, name="on")
            nc.sync.dma_start(out=on_sb, in_=omnsq[:, :])
            mk_sb = big.tile([128, 128], F16, name="mk")
            nc.sync.dma_start(out=mk_sb, in_=maskT[:, :])
            id_sb = big.tile([128, 128], F16, name="id")
            nc.sync.dma_start(out=id_sb, in_=ident[:, :])
            bqk_sb = []
            for mi in range(4):
                t_ = big.tile([128, 1], F32, name=f"bqk{mi}")
                nc.sync.dma_start(out=t_, in_=bqk[mi * 128:(mi + 1) * 128, :])
                bqk_sb.append(t_)
            bv_sb = big.tile([1, HPC * D], F16, name="bv")
            nc.sync.dma_start(out=bv_sb, in_=bv[:, :])
            ones_r = big.tile([1, 128], F16, name="ones_r")
            nc.vector.memset(ones_r, 1.0)
            lnsc_sb = big.tile([128, 1], F32, name="lnsc")
            nc.vector.memset(lnsc_sb, LN_SCALE)
            nhm_sb = big.tile([128, 1], F32, name="nhm")
            nc.vector.memset(nhm_sb, NEG_HALF_LN_M)

            wqkall = big.tile([128, NK * 512], F16, name="wqkall")
            nc.sync.dma_start(
                out=wqkall[:, :].rearrange("p (a n) -> p a n", a=NK),
                in_=wqk[:, :].rearrange("(a p) n -> p a n", p=128))
            wqk_sb = [wqkall[:, ki * 512:(ki + 1) * 512] for ki in range(NK)]
            xtall = big.tile([128, NK * T], F16, name="xtall")
            nc.scalar.dma_start(
                out=xtall[:, :].rearrange("p (a t) -> p a t", a=NK),
                in_=xT[:, :].rearrange("(a p) t -> p a t", p=128))
            xt_sb = [xtall[:, ki * T:(ki + 1) * T] for ki in range(NK)]
            wvall = big.tile([128, NK * HPC * D], F16, name="wvall")
            nc.sync.dma_start(
                out=wvall[:, :].rearrange("p (a n) -> p a n", a=NK),
                in_=wv[:, :].rearrange("(a p) n -> p a n", p=128))
            wv_sb = [wvall[:, ki * HPC * D:(ki + 1) * HPC * D]
                     for ki in range(NK)]
            wpall = big.tile([128, 2 * C], F16, name="wpall")
            nc.scalar.dma_start(
                out=wpall[:, :].rearrange("p (a n) -> p a n", a=2),
                in_=wp[:, :].rearrange("(a p) n -> p a n", p=128))
            wp_sb = [wpall[:, ci2 * C:(ci2 + 1) * C] for ci2 in range(2)]

            # ---- persistent intermediates ----
            qt_sb = [big.tile([128, T], F16, name=f"qt{j}") for j in range(2)]
            ktsq_sb = [big.tile([128, T], F16, name=f"ktsq{h}") for h in range(HPC)]
            eq_sb = [big.tile([128, T], F16, name=f"eq{h}") for h in range(HPC)]
            ekt_sb = [big.tile([128, T], F16, name=f"ekt{h}") for h in range(HPC)]
            v_sb = [big.tile([128, HPC * (D + 1)], F16, name=f"v{ti}")
                    for ti in range(NT)]
            yt_sb = [big.tile([128, T], F16, name=f"yt{j}") for j in range(2)]

            # ---- phase 1: k then q (transposed layout) ----
            for mi in (2, 3, 0, 1):      # k head pairs first, then q
                for ni in range(2):
                    tsl = slice(ni * 512, (ni + 1) * 512)
                    p_ = bankA()
                    for ki in range(NK):
                        nc.tensor.matmul(
                            p_[:, :],
                            wqk_sb[ki][:, mi * 128:(mi + 1) * 128],
                            xt_sb[ki][:, tsl],
                            start=(ki == 0), stop=(ki == NK - 1))
                    if mi < 2:
                        nc.vector.tensor_scalar_add(
                            qt_sb[mi][:, tsl], p_[:, :], bqk_sb[mi])
                    else:
                        for par in range(2):
                            h = (mi - 2) * 2 + par
                            rs = par * 64
                            nc.vector.tensor_scalar_add(
                                ktsq_sb[h][0:64, tsl], p_[rs:rs + 64, :],
                                bqk_sb[mi][rs:rs + 64, :])
                            nc.vector.tensor_mul(
                                ktsq_sb[h][64:128, tsl],
                                ktsq_sb[h][0:64, tsl],
                                ktsq_sb[h][0:64, tsl])

            # ---- phase 1b: v (natural layout, ones col per head) ----
            for ti in range(NT):
                nc.vector.memset(
                    v_sb[ti][:, :].rearrange("p (h c) -> p h c", c=D + 1)
                    [:, :, D:D + 1], 1.0)
                p_ = bankA()
                for ki in range(NK):
                    nc.tensor.matmul(
                        p_[:, 0:HPC * D],
                        xt_sb[ki][:, ti * 128:(ti + 1) * 128],
                        wv_sb[ki][:, :],
                        start=(ki == 0), stop=False)
                nc.tensor.matmul(p_[:, 0:HPC * D], ones_r[:, :], bv_sb[:, :],
                                 start=False, stop=True)
                nc.scalar.copy(
                    v_sb[ti][:, :].rearrange("p (h c) -> p h c", c=D + 1)
                    [:, :, 0:D],
                    p_[:, 0:HPC * D].rearrange("p (h c) -> p h c", c=D))

            # ---- phase 2: Eq, EkT per head ----
            for h in range(HPC):
                mi, rs = h // 2, (h % 2) * 64
                for ni in range(2):
                    tsl = slice(ni * 512, (ni + 1) * 512)
                    pq = bankA()
                    nc.tensor.matmul(pq[:, :], om_sb[rs:rs + 64, :],
                                     qt_sb[mi][rs:rs + 64, tsl],
                                     start=True, stop=True)
                    nc.scalar.activation(eq_sb[h][:, tsl], pq[:, :], Exp,
                                         bias=lnsc_sb[:, :], scale=1.0)
                    pk2 = bankA()
                    nc.tensor.matmul(pk2[:, :], om_sb[0:64, :],
                                     ktsq_sb[h][0:64, tsl],
                                     start=True, stop=True)
                    nc.scalar.activation(ekt_sb[h][:, tsl], pk2[:, :], Exp,
                                         bias=lnsc_sb[:, :], scale=1.0)

            # ---- phase 3: chunked FAVOR, head pairs interleaved ----
            for pair in range(2):
                heads = (2 * pair, 2 * pair + 1)
                s_tiles = {h: chk.tile([128, D + 1], F16, name=f"S{h % 2}")
                           for h in heads}
                ps_s = {h: ps.tile([128, D + 1], F32, name="psS", bufs=2)
                        for h in heads}
                for ci in range(NT):
                    csl = slice(ci * L, (ci + 1) * L)
                    for h in heads:
                        mi, rs = h // 2, (h % 2) * 64
                        vsl = slice(h * (D + 1), (h + 1) * (D + 1))
                        s_sb = s_tiles[h]
                        # [projk | -nsq] in one matmul
                        pk = ps.tile([128, 129], F32, name="pk", bufs=1)
                        nc.tensor.matmul(pk[:, :], ktsq_sb[h][:, csl],
                                         on_sb[:, :], start=True, stop=True)
                        fkc = col.tile([128, 1], F32, name="fkc")
                        nc.scalar.activation(fkc, pk[:, 128:129], Exp,
                                             bias=nhm_sb[:, :], scale=1.0)
                        ekh = chk.tile([128, 128], F16, name="ekh")
                        nc.scalar.activation(ekh, pk[:, 0:128], Exp,
                                             bias=lnsc_sb[:, :], scale=1.0)
                        vh = chk.tile([128, D + 1], F16, name="vh")
                        nc.vector.tensor_scalar_mul(vh, v_sb[ci][:, vsl], fkc)
                        # A_T, masked + fk row scale
                        pA = ps.tile([128, 128], F32, name="pA", bufs=1)
                        nc.tensor.matmul(pA[:, :], ekt_sb[h][:, csl],
                                         eq_sb[h][:, csl],
                                         start=True, stop=True)
                        atm = chk.tile([128, 128], F16, name="atm")
                        nc.vector.scalar_tensor_tensor(
                            atm, pA[:, :], fkc, mk_sb[:, :],
                            op0=mybir.AluOpType.mult,
                            op1=mybir.AluOpType.mult)
                        # num/den
                        pY = ps.tile([128, D + 1], F32, name="pYt", bufs=2)
                        if ci > 0:
                            nc.tensor.matmul(pY[:, :], eq_sb[h][:, csl],
                                             s_sb[:, :],
                                             start=True, stop=False)
                        nc.tensor.matmul(pY[:, :], atm, v_sb[ci][:, vsl],
                                         start=(ci == 0), stop=True)
                        # y = num/den
                        rc = col.tile([128, 1], F32, name="rc")
                        nc.vector.reciprocal(rc, pY[:, D:D + 1])
                        ych = chk.tile([128, D], F16, name="ych")
                        nc.scalar.activation(
                            ych, pY[:, 0:D],
                            mybir.ActivationFunctionType.Copy,
                            bias=0.0, scale=rc[:, :])
                        # yT via PE transpose
                        pyt = ps.tile([64, 128], F16, name="pYt", bufs=2)
                        nc.tensor.transpose(pyt[:, :], ych[:, :], id_sb[:, :])
                        nc.vector.tensor_copy(yt_sb[mi][rs:rs + 64, csl],
                                              pyt[:, :])
                        # state update (after the inter-chunk read of s_sb)
                        nc.tensor.matmul(ps_s[h][:, :], ekh, vh[:, :],
                                         start=(ci == 0), stop=(ci == NT - 1),
                                         skip_group_check=True)
                        if ci < NT - 1:
                            nc.scalar.copy(s_sb[:, :], ps_s[h][:, :])

            # ---- phase 4: output projection ----
            for ti in range(NT):
                for ni in range(2):
                    nsl = slice(ni * 512, (ni + 1) * 512)
                    pp = bankA()
                    for ci2 in range(2):
                        nc.tensor.matmul(pp[:, :],
                                         yt_sb[ci2][:, ti * 128:(ti + 1) * 128],
                                         wp_sb[ci2][:, nsl],
                                         start=(ci2 == 0), stop=(ci2 == 1))
                    osb = cpy.tile([128, 512], F32, name="osb")
                    if (2 * ti + ni) % 4 == 3:
                        nc.vector.tensor_copy(osb[:, :], pp[:, :])
                    else:
                        nc.scalar.copy(osb[:, :], pp[:, :])
                    nc.sync.dma_start(
                        out=outp[ti * 128:(ti + 1) * 128, nsl], in_=osb[:, :])

    _split_waits(nc)
    return nc


_NC_CACHE = None


def _get_nc():
    global _NC_CACHE
    if _NC_CACHE is None:
        _NC_CACHE = build_bass()
    return _NC_CACHE


def kernel(x, W_attn, b_attn, W_proj, b_proj, omega):
    from concourse.bass_utils import run_bass_kernel_spmd

    x = np.asarray(x, dtype=np.float32)
    W_attn = np.asarray(W_attn, dtype=np.float32)
    b_attn = np.asarray(b_attn, dtype=np.float32)
    W_proj = np.asarray(W_proj, dtype=np.float32)
    b_proj = np.asarray(b_proj, dtype=np.float32)
    omega = np.asarray(omega, dtype=np.float32)

    B = x.shape[0]
    scale = 1.0 / math.sqrt(D)
    omega2 = np.concatenate([omega, omega], axis=0).astype(np.float16)
    omnsq = np.zeros((128, 129), np.float32)
    omnsq[0:64, 0:128] = omega
    omnsq[64:128, 128] = -0.5
    omnsq = omnsq.astype(np.float16)
    maskT = np.triu(np.ones((128, 128), np.float32)).astype(np.float16)
    ident = np.eye(128, dtype=np.float16)
    xTs = [np.ascontiguousarray(x[b].T).astype(np.float16) for b in range(B)]

    in_maps = []
    for core in range(8):
        b, g = core // 4, core % 4
        ch0 = g * HPC * D
        wq = W_attn[:, ch0:ch0 + HPC * D] * scale
        wk = W_attn[:, C + ch0:C + ch0 + HPC * D] * scale
        wqk_ = np.concatenate([wq, wk], axis=1).astype(np.float16)
        wv_ = W_attn[:, 2 * C + ch0:2 * C + ch0 + HPC * D].astype(np.float16)
        bqk_ = (np.concatenate([b_attn[ch0:ch0 + HPC * D],
                                b_attn[C + ch0:C + ch0 + HPC * D]]) * scale
                ).astype(np.float32).reshape(-1, 1)
        bv_ = b_attn[2 * C + ch0:2 * C + ch0 + HPC * D].astype(
            np.float16).reshape(1, -1)
        wp_ = W_proj[ch0:ch0 + HPC * D, :].astype(np.float16)
        in_maps.append({
            "xT": xTs[b], "wqk": wqk_, "wv": wv_, "wp": wp_,
            "omega2": omega2, "omnsq": omnsq, "maskT": maskT, "ident": ident,
            "bqk": bqk_, "bv": bv_,
        })

    nc = _get_nc()
    res = run_bass_kernel_spmd(nc, in_maps, list(range(8)))

    out = np.zeros((B, T, C), dtype=np.float32)
    for core in range(8):
        out[core // 4] += res.results[core]["outp"]
    out += b_proj[None, None, :]
    return out
